# revision 14
# baseline (speedup 1.0000x reference)
"""Multi-head attention (B=2, S=2048, D=1024, H=16, causal) on 8 trn2 cores.

Sharding: 2 batches x 4 head-groups (4 heads / 256 proj dims per core).

Numerics on TRN2 PE: "fp32" matmul has only ~12-bit product precision
(~2.5e-4), same as fp32r. For the attention-probability pipeline
(q/k projections + logits) we therefore use an exact double-bf16 3-pass
decomposition (x = hi + lo in bf16; A@B ~ Ahi@Bhi + Ahi@Blo + Alo@Bhi,
error ~3e-5). The value path (v-proj, attn@V, dense) uses fp32r
(1 cycle/row) whose ~2.5e-4 product error is acceptable there.

Per core:
  - q/k projections in transposed layout [dout, s] as bf16 hi/lo pairs
  - v projection in natural layout [s, dout] as fp32r, with a ones column
    appended per head so attn@V also yields the softmax row sums
  - causal attention per head: logits computed in both orientations
    ([i,j] for the attn output, [j,i] for the attn@V contraction) to avoid
    on-chip transposes; softmax without max-subtraction (logits ~ N(0,1),
    exp is safe; masked entries produce exactly 0.0 like the reference)
  - dense projection partial product; host reduces partials over groups
The strict upper triangle of attn is never written: ExternalOutput buffers
are donated pre-zeroed under PJRT, so it stays exactly 0.0.
"""

import numpy as np

B, S, D, H = 2, 2048, 1024, 16
G = 4                 # head-group shards (tensor parallel)
HL = H // G           # heads per core = 4
DEPTH = D // H        # 64
DG = D // G           # proj dims per core = 256
NCORES = 8
NT = S // 128         # 16 row tiles
NC4 = 4               # 512-wide column chunks
SCALE = 1.0 / np.sqrt(np.float32(DEPTH))   # 0.125

_prog_cache = {}


def _split_sync_waits(nc):
    """The walrus build in this container allows only one semaphore wait per
    instruction ("Too many sync wait commands"). Split every multi-wait
    instruction: keep the last wait on the instruction and prepend
    same-engine NoOps carrying the others."""
    import concourse.mybir as mybir

    n = 0
    for func in nc.m.functions:
        for blk in func.blocks:
            insts = blk.instructions
            out = []
            for inst in insts:
                si = getattr(inst, "sync_info", None)
                waits = list(si.on_wait) if si is not None and si.on_wait else []
                if len(waits) > 1:
                    for w in waits[:-1]:
                        n += 1
                        nop = mybir.InstNoOp(
                            name=f"I-waitsplit-{n}", engine=inst.engine,
                            ins=[], outs=[],
                            sync_info=mybir.SyncInfo(on_wait=[w], on_update=[]),
                        )
                        out.append(nop)
                    inst.sync_info = mybir.SyncInfo(
                        on_wait=[waits[-1]],
                        on_update=list(si.on_update or []),
                    )
                out.append(inst)
            insts[:] = out


def _build_program():
    import concourse.bass as bass
    import concourse.tile as tile
    from concourse import mybir

    F32 = mybir.dt.float32
    F32R = mybir.dt.float32r
    BF16 = mybir.dt.bfloat16
    EXP = mybir.ActivationFunctionType.Exp
    ADD = mybir.AluOpType.add
    MULT = mybir.AluOpType.mult
    SUB = mybir.AluOpType.subtract

    nc = bass.Bass()
    dp = nc.declare_dram_parameter
    d_qhi = dp("qhi", [D, S], BF16, isOutput=False)
    d_qlo = dp("qlo", [D, S], BF16, isOutput=False)
    d_khi = dp("khi", [D, S], BF16, isOutput=False)
    d_klo = dp("klo", [D, S], BF16, isOutput=False)
    d_vr = dp("vr", [D, S], F32R, isOutput=False)
    d_wqhi = dp("wqhi", [D, DG], BF16, isOutput=False)
    d_wqlo = dp("wqlo", [D, DG], BF16, isOutput=False)
    d_wkhi = dp("wkhi", [D, DG], BF16, isOutput=False)
    d_wklo = dp("wklo", [D, DG], BF16, isOutput=False)
    d_wvr = dp("wvr", [D, DG], F32R, isOutput=False)
    d_dwr = dp("dwr", [DG, D], F32R, isOutput=False)
    d_bq = dp("bq", [128, 2], F32, isOutput=False)
    d_bk = dp("bk", [128, 2], F32, isOutput=False)
    d_triu = dp("triu", [128, 128], F32, isOutput=False)
    d_tri01 = dp("tri01", [128, 896], F32, isOutput=False)
    d_eye = dp("eye", [128, 128], F32, isOutput=False)
    d_attn = dp("attn", [HL, S, S], F32, isOutput=True)
    d_outp = dp("outp", [S, D], F32, isOutput=True)

    with tile.TileContext(nc) as tc:
        with (
            tc.tile_pool(name="consts", bufs=1) as consts,
            tc.tile_pool(name="proj", bufs=1) as proj,
        ):
            # ---- constants ----
            triu = consts.tile([128, 128], F32)
            nc.sync.dma_start(triu[:], d_triu[:])
            tri01 = consts.tile([128, 896], F32)
            nc.sync.dma_start(tri01[:], d_tri01[:])
            eye = consts.tile([128, 128], F32)
            nc.sync.dma_start(eye[:], d_eye[:])
            bq = consts.tile([128, 2], F32)
            nc.sync.dma_start(bq[:], d_bq[:])
            bk = consts.tile([128, 2], F32)
            nc.sync.dma_start(bk[:], d_bk[:])
            ones1 = consts.tile([1, 128], F32)
            nc.vector.memset(ones1[:], 1.0)
            wvr = consts.tile([128, 8, DG], F32R)
            nc.sync.dma_start(wvr[:], d_vrearr(d_wvr))
            dw = [consts.tile([64, D], F32R, tag=f"dw{h}", name=f"dw{h}")
                  for h in range(HL)]
            for h in range(HL):
                nc.sync.dma_start(dw[h][:], d_dwr[h * 64:(h + 1) * 64, :])

            # ---- persistent projection outputs ----
            Qhi = [proj.tile([128, S], BF16, tag=f"Qhi{m}", name=f"Qhi{m}")
                   for m in range(2)]
            Qlo = [proj.tile([128, S], BF16, tag=f"Qlo{m}", name=f"Qlo{m}")
                   for m in range(2)]
            Khi = [proj.tile([128, S], BF16, tag=f"Khi{m}", name=f"Khi{m}")
                   for m in range(2)]
            Klo = [proj.tile([128, S], BF16, tag=f"Klo{m}", name=f"Klo{m}")
                   for m in range(2)]
            V = proj.tile([128, NT, HL, DEPTH + 1], F32R, tag="V")
            # ones everywhere; the v-projection copies overwrite cols 0:DEPTH,
            # leaving a ones column at index DEPTH per head (row-sum trick)
            nc.vector.memset(V[:, :, :, :].bitcast(F32), 1.0)
            rc = [proj.tile([128, NT], F32, tag=f"rc{h}", name=f"rc{h}")
                  for h in range(HL)]

            # ---- stage A: projections ----
            with (
                tc.tile_pool(name="wts", bufs=1) as wts,
                tc.tile_pool(name="tmpf", bufs=2) as tmpf,
                tc.tile_pool(name="pproj", bufs=2,
                             space=bass.MemorySpace.PSUM) as pproj,
            ):
                whi = {}
                wlo = {}
                for key, dhi, dlo in (("q", d_wqhi, d_wqlo),
                                      ("k", d_wkhi, d_wklo)):
                    whi[key] = wts.tile([128, 8, DG], BF16, tag=f"whi{key}", name=f"whi{key}")
                    nc.sync.dma_start(whi[key][:], d_vrearr(dhi))
                    wlo[key] = wts.tile([128, 8, DG], BF16, tag=f"wlo{key}", name=f"wlo{key}")
                    nc.sync.dma_start(wlo[key][:], d_vrearr(dlo))

                for key, d_ihi, d_ilo, OH, OL, bias, scale in (
                    ("q", d_qhi, d_qlo, Qhi, Qlo, bq, float(SCALE)),
                    ("k", d_khi, d_klo, Khi, Klo, bk, 1.0),
                ):
                    # all 8 k-tiles resident (accumulation is kt-innermost)
                    with tc.tile_pool(name=f"ins{key}", bufs=1) as ins:
                        ihi, ilo = [], []
                        for kt in range(8):
                            th = ins.tile([128, S], BF16, tag=f"ih{kt}")
                            nc.sync.dma_start(
                                th[:], d_ihi[kt * 128:(kt + 1) * 128, :])
                            ihi.append(th)
                            tl = ins.tile([128, S], BF16, tag=f"il{kt}")
                            nc.sync.dma_start(
                                tl[:], d_ilo[kt * 128:(kt + 1) * 128, :])
                            ilo.append(tl)
                        for m in range(2):
                            for sb in range(4):
                                ss = slice(sb * 512, (sb + 1) * 512)
                                ps = pproj.tile([128, 512], F32, tag="pp")
                                for kt in range(8):
                                    wh = whi[key][:, kt, m * 128:(m + 1) * 128]
                                    wl = wlo[key][:, kt, m * 128:(m + 1) * 128]
                                    nc.tensor.matmul(
                                        ps[:], wh, ihi[kt][:, ss],
                                        start=(kt == 0), stop=False)
                                    nc.tensor.matmul(
                                        ps[:], wh, ilo[kt][:, ss],
                                        start=False, stop=False)
                                    nc.tensor.matmul(
                                        ps[:], wl, ihi[kt][:, ss],
                                        start=False, stop=(kt == 7))
                                qf = tmpf.tile([128, 512], F32, tag="qf")
                                nc.vector.tensor_scalar(
                                    qf[:], ps[:], scale, bias[:, m:m + 1],
                                    op0=MULT, op1=ADD)
                                nc.vector.tensor_copy(OH[m][:, ss], qf[:])
                                nc.vector.tensor_tensor(
                                    OL[m][:, ss], qf[:], OH[m][:, ss], op=SUB)

                # v projection (natural orientation, fp32r)
                with tc.tile_pool(name="insv", bufs=1) as insv:
                    iv = []
                    for kt in range(8):
                        tv = insv.tile([128, S], F32R, tag=f"iv{kt}")
                        nc.sync.dma_start(
                            tv[:], d_vr[kt * 128:(kt + 1) * 128, :])
                        iv.append(tv)
                    for st in range(NT):
                        ps = pproj.tile([128, 512], F32, tag="pp")
                        for kt in range(8):
                            nc.tensor.matmul(
                                ps[:, 0:DG],
                                iv[kt][:, st * 128:(st + 1) * 128],
                                wvr[:, kt, :],
                                start=(kt == 0), stop=(kt == 7))
                        nc.vector.tensor_copy(
                            V[:, st, :, 0:DEPTH],
                            ps[:, 0:DG].rearrange("p (h e) -> p h e", h=HL))

            # ---- stage B/C: attention + dense, chunk by chunk ----
            with (
                tc.tile_pool(name="psp", bufs=3,
                             space=bass.MemorySpace.PSUM) as psp,
                tc.tile_pool(name="pav", bufs=2,
                             space=bass.MemorySpace.PSUM) as pav,
                tc.tile_pool(name="arowp", bufs=2) as arowp,
                tc.tile_pool(name="eTp", bufs=2) as eTp,
                tc.tile_pool(name="eTrp", bufs=2) as eTrp,
                tc.tile_pool(name="ctp", bufs=2) as ctp,
                tc.tile_pool(name="smallp", bufs=2) as smallp,
                tc.tile_pool(name="orp", bufs=2) as orp,
            ):
                for c in range(NC4):
                    ii = slice(c * 512, (c + 1) * 512)
                    CTc = [ctp.tile([64, 512], F32R, tag=f"ct{h}", name=f"ct{h}_{c}")
                           for h in range(HL)]
                    for pair in range(2):
                        m = pair
                        njt = 4 * c + 4
                        av = [pav.tile([65, 512], F32, tag="av", name=f"av{hx}_{pair}_{c}")
                              for hx in range(2)]
                        # --- [j, i] logits + exp + attn @ V ---
                        for jt in range(njt):
                            jj = slice(jt * 128, (jt + 1) * 128)
                            ps = psp.tile([128, 2, 512], F32, tag="ps")
                            for hs in range(2):
                                r = slice(hs * 64, hs * 64 + 64)
                                nc.tensor.matmul(ps[:, hs, :], Khi[m][r, jj],
                                                 Qhi[m][r, ii],
                                                 start=True, stop=False)
                                nc.tensor.matmul(ps[:, hs, :], Khi[m][r, jj],
                                                 Qlo[m][r, ii],
                                                 start=False, stop=False)
                                nc.tensor.matmul(ps[:, hs, :], Klo[m][r, jj],
                                                 Qhi[m][r, ii],
                                                 start=False, stop=True)
                            eT = eTp.tile([128, 2, 512], F32, tag="eT")
                            nc.scalar.activation(eT[:], ps[:], EXP)
                            eTr = eTrp.tile([128, 2, 512], F32R, tag="eTr")
                            if jt >= 4 * c:
                                o = (jt - 4 * c) * 128
                                msk = tri01[:, 384 - o:896 - o].unsqueeze(1) \
                                    .broadcast_to([128, 2, 512])
                                nc.vector.tensor_tensor(eTr[:], eT[:], msk,
                                                        op=MULT)
                            else:
                                nc.vector.tensor_copy(eTr[:], eT[:])
                            for hs in range(2):
                                h = 2 * pair + hs
                                nc.tensor.matmul(
                                    av[hs][:], V[:, jt, h, :], eTr[:, hs, :],
                                    start=(jt == 0), stop=(jt == njt - 1))

                        # --- per-head: normalize ctx, reciprocals ---
                        for hs in range(2):
                            h = 2 * pair + hs
                            cu = smallp.tile([64, 512], F32, tag="cu")
                            nc.vector.tensor_copy(cu[:], av[hs][0:64, :])
                            sr = smallp.tile([65, 512], F32, tag="sr")
                            nc.vector.tensor_copy(sr[64:65, :],
                                                  av[hs][64:65, :])
                            # move the sums row to partition 0 (DMA can
                            # cross partitions; engines cannot)
                            s0 = smallp.tile([1, 512], F32, tag="s0")
                            nc.sync.dma_start(s0[:], sr[64:65, :])
                            rrow = smallp.tile([1, 512], F32, tag="rrow")
                            nc.vector.reciprocal(rrow[:], s0[:])
                            # broadcast across partitions via ones matmul
                            # (1.0 x r is exact on the PE)
                            ps = psp.tile([128, 2, 512], F32, tag="ps")
                            nc.tensor.matmul(ps[:, 0, :], ones1[:], rrow[:])
                            bi = smallp.tile([128, 512], F32, tag="bi")
                            nc.vector.tensor_copy(bi[:], ps[:, 0, :])
                            nc.vector.tensor_tensor(CTc[h][:, :], cu[:],
                                                    bi[0:64, :], op=MULT)
                            pc = pav.tile([128, 128], F32, tag="av")
                            for tt in range(4):
                                nc.tensor.transpose(
                                    pc[:, :],
                                    bi[:, tt * 128:(tt + 1) * 128], eye[:])
                                nc.vector.tensor_copy(
                                    rc[h][:, 4 * c + tt:4 * c + tt + 1],
                                    pc[:, 0:1])

                        # --- [i, j] attn output rows for this chunk ---
                        for t in range(4 * c, 4 * c + 4):
                            W = (t + 1) * 128
                            tsl = slice(t * 128, (t + 1) * 128)
                            arow = arowp.tile([128, 2, S], F32, tag="arow")
                            for jc in range(0, W, 512):
                                w = min(512, W - jc)
                                jj = slice(jc, jc + w)
                                ps = psp.tile([128, 2, 512], F32, tag="ps")
                                for hs in range(2):
                                    r = slice(hs * 64, hs * 64 + 64)
                                    nc.tensor.matmul(
                                        ps[:, hs, 0:w], Qhi[m][r, tsl],
                                        Khi[m][r, jj], start=True, stop=False)
                                    nc.tensor.matmul(
                                        ps[:, hs, 0:w], Qhi[m][r, tsl],
                                        Klo[m][r, jj], start=False, stop=False)
                                    nc.tensor.matmul(
                                        ps[:, hs, 0:w], Qlo[m][r, tsl],
                                        Khi[m][r, jj], start=False, stop=True)
                                if jc + w == W:
                                    mu = triu[:].unsqueeze(1) \
                                        .broadcast_to([128, 2, 128])
                                    nc.vector.tensor_tensor(
                                        ps[:, :, w - 128:w],
                                        ps[:, :, w - 128:w], mu, op=ADD)
                                nc.scalar.activation(
                                    arow[:, :, jc:jc + w], ps[:, :, 0:w], EXP)
                            for hs in range(2):
                                h = 2 * pair + hs
                                nc.vector.tensor_scalar_mul(
                                    arow[:, hs, 0:W], arow[:, hs, 0:W],
                                    rc[h][:, t:t + 1])
                                nc.sync.dma_start(
                                    d_attn[h, tsl, 0:W], arow[:, hs, 0:W])

                    # --- dense partial for this chunk's four s-tiles ---
                    for tl, st in enumerate(range(4 * c, 4 * c + 4)):
                        orow = orp.tile([128, D], F32, tag="orow")
                        for nb in range(2):
                            ps = psp.tile([128, 2, 512], F32, tag="ps")
                            pd = ps[:, 0, :]
                            for h in range(HL):
                                nc.tensor.matmul(
                                    pd, CTc[h][:, tl * 128:(tl + 1) * 128],
                                    dw[h][:, nb * 512:(nb + 1) * 512],
                                    start=(h == 0), stop=(h == HL - 1))
                            nc.vector.tensor_copy(
                                orow[:, nb * 512:(nb + 1) * 512], pd)
                        nc.sync.dma_start(
                            d_outp[st * 128:(st + 1) * 128, :], orow[:])

    _split_sync_waits(nc)
    return nc


def d_vrearr(dram_ap_handle):
    """[D, C] dram tensor viewed as [128, 8, C] (kt-major along free)."""
    return dram_ap_handle[:].rearrange("(t p) c -> p t c", p=128)


def _round_fp32r(x):
    """Round fp32 to fp32r: nearest-even at 12-bit mantissa granularity
    (matches neuronxcc static_cast_fp32_to_fp32r)."""
    u = np.ascontiguousarray(x, dtype=np.float32).view(np.uint32).astype(np.uint64)
    low = u & np.uint64(0xFFF)
    base = u & np.uint64(0xFFFFF000)
    up = base + np.uint64(0x1000)
    tie_up = (low == 0x800) & (((u >> np.uint64(12)) & np.uint64(1)) == 1)
    rounded = np.where((low > 0x800) | tie_up, up, base)
    return rounded.astype(np.uint32).view(np.float32).reshape(x.shape)


def _split_bf16(x):
    import ml_dtypes
    hi = x.astype(ml_dtypes.bfloat16)
    lo = (x - hi.astype(np.float32)).astype(ml_dtypes.bfloat16)
    return np.ascontiguousarray(hi), np.ascontiguousarray(lo)


def _host_inputs(q, k, v, mask, wq_w, wq_b, wk_w, wk_b, wv_w, wv_b,
                 dense_w, dense_b):
    f32 = np.float32
    triu = (np.triu(np.ones((128, 128), f32), k=1) * f32(-1e9)).astype(f32)
    # tri01[p, z] = 1.0 where (z - 384) >= p else 0.0 (keep i >= j)
    z = np.arange(896)[None, :] - 384
    p = np.arange(128)[:, None]
    tri01 = np.where(z >= p, f32(1.0), f32(0.0)).astype(f32)
    eye = np.eye(128, dtype=f32)

    qs = [_split_bf16(np.ascontiguousarray(q[b].T)) for b in range(B)]
    ks = [_split_bf16(np.ascontiguousarray(k[b].T)) for b in range(B)]
    vs = [_round_fp32r(np.ascontiguousarray(v[b].T)) for b in range(B)]

    in_maps = []
    for core in range(NCORES):
        b, g = divmod(core, G)
        sl = slice(g * DG, (g + 1) * DG)
        wq_hi, wq_lo = _split_bf16(np.ascontiguousarray(wq_w[sl, :].T))
        wk_hi, wk_lo = _split_bf16(np.ascontiguousarray(wk_w[sl, :].T))
        in_maps.append({
            "qhi": qs[b][0], "qlo": qs[b][1],
            "khi": ks[b][0], "klo": ks[b][1],
            "vr": vs[b],
            "wqhi": wq_hi, "wqlo": wq_lo,
            "wkhi": wk_hi, "wklo": wk_lo,
            "wvr": _round_fp32r(np.ascontiguousarray(wv_w[sl, :].T)),
            "dwr": _round_fp32r(np.ascontiguousarray(dense_w[:, sl].T)),
            "bq": np.ascontiguousarray(
                (wq_b[sl] * SCALE).reshape(2, 128).T).astype(f32),
            "bk": np.ascontiguousarray(wk_b[sl].reshape(2, 128).T).astype(f32),
            "triu": triu, "tri01": tri01, "eye": eye,
        })
    return in_maps


def kernel(q, k, v, mask, wq_w, wq_b, wk_w, wk_b, wv_w, wv_b,
           dense_w, dense_b, _results_hook=None):
    import os
    from concourse.bass_utils import run_bass_kernel_spmd

    args = [np.ascontiguousarray(np.asarray(x, dtype=np.float32)) for x in
            (q, k, v, mask, wq_w, wq_b, wk_w, wk_b, wv_w, wv_b,
             dense_w, dense_b)]
    (q, k, v, mask, wq_w, wq_b, wk_w, wk_b, wv_w, wv_b,
     dense_w, dense_b) = args

    if "nc" not in _prog_cache:
        _prog_cache["nc"] = _build_program()
    nc = _prog_cache["nc"]

    kw = {}
    if os.environ.get("ATTN_KERNEL_TRACE"):
        kw = dict(trace=True, tmpdir=os.environ.get("ATTN_KERNEL_TRACE_DIR"))
    in_maps = _host_inputs(q, k, v, mask, wq_w, wq_b, wk_w, wk_b, wv_w, wv_b,
                           dense_w, dense_b)
    res = run_bass_kernel_spmd(nc, in_maps, list(range(NCORES)), **kw)
    if _results_hook is not None:
        _results_hook(res)

    attn = np.empty((B, H, S, S), np.float32)
    out = np.empty((B, S, D), np.float32)
    for b in range(B):
        acc = np.zeros((S, D), np.float64)
        for g in range(G):
            r = res.results[b * G + g]
            attn[b, g * HL:(g + 1) * HL] = r["attn"]
            acc += r["outp"].astype(np.float64)
            sl = slice(g * DG, (g + 1) * DG)
            acc += (wv_b[sl].astype(np.float64)
                    @ dense_w[:, sl].T.astype(np.float64))
        out[b] = (acc + dense_b.astype(np.float64)).astype(np.float32)
    return out, attn
